# revision 18
# baseline (speedup 1.0000x reference)
"""Causal (running) weighted mean/std scaler for Trainium2 (Bass/Tile).

Full inputs: data/padding_mask/weights [16, 256, 8192]; outputs
(scaled_data, causal_means, causal_scale) as f32 [16, 256, 8192]
(scaled/scale computed and stored in fp16, upcast on gather).

Sharding: fully data-parallel along B*V (4096 rows) across 8 NeuronCores,
512 rows per core; cumulative sums run along T, which stays local.

Design: the three running sums (cum_w, cum_wx, cum_s) run on the
otherwise-idle PE (tensor) engine as triangular matmuls instead of the
DVE tensor_tensor_scan recurrence (2 cyc/elem) that bounded the previous
kernel at ~433 us.  Per 256-wide subchunk, PE-transposed data blocks are
the stationary operand and sliding windows of a constant
[zeros | upper-tri | ones] mask are the moving operand; 128-block partial
cumsums accumulate in PSUM, and running carries across subchunks are
[128,1] columns folded into the consumers (ACT bias APs /
scalar_tensor_tensor scalar operands), so no partition-broadcast or
extra passes are needed.

Precision: the mean path needs exact sums before ~512 accumulated terms
(no statistical cancellation of rounding noise early on, and mean errors
are amplified by 1/scale against the graded rel-err floor), so the first
chunk of every row-panel uses fp32 matmuls; later chunks use fp16
stationaries (8x finer mantissa than bf16 at the same 1 cyc/row PE rate).
The variance path is robust (positive sums + sqrt(var+0.1) floor) and
runs fp16 throughout.  Outputs scaled/scale are stored fp16 (within the
2e-2 tolerance; cuts store traffic by a third); means must stay f32.

Emission is phase-grouped across the four row-panels (P1 sums/mean,
P2 residual/variance, P3 outputs) with chunk-ahead DMA prefetch:
per-engine queues are FIFO, so emitting one chunk's full cross-engine
chain in order head-of-line-blocks every engine on the chain's stalls;
phase grouping overlaps panel i's stall with panel j's work.  Measured
(hw-loop rep-differential): ~290-390 us steady-state vs 433-505 us for
the scan-based baseline; worst graded rel err 1.14e-2 (< 2e-2), binding
constraint is fp16 summation noise in cum_wx at near-zero means.
"""

from contextlib import ExitStack

import numpy as np

B, V, T = 16, 256, 8192
NCORES = 8
ROWS = B * V // NCORES  # 512
W = 1024  # elementwise chunk width
MINIMUM_SCALE = 0.1

_CACHE = {}


def _build_nc(
    rows, t, w_chunk=W, sub=256, reps=1, hw_reps=0, bench_internal=False,
    bf16_out=True, copy_split=False, sc_on_dve=False, r2_on_dve=False,
    w_on_pool=False, dma_only=False, no_stores=False, in_bufs=3,
    tr_bufs=2, mm_bufs=2, interleave=True, f32_chunks=1, pool_tiny=False,
    eb=5, share_mmAC=False, sd_on_pool=False, dma_tr_r2=False, dma_tr_w=False,
    carbc_act=False, lninv=False, dpcar=False, mmB_extra=0,
):
    import concourse.bass as bass
    import concourse.tile as tile
    from concourse import bacc, mybir
    from concourse.masks import make_identity, make_upper_triangular

    f32 = mybir.dt.float32
    i32 = mybir.dt.int32
    bf16 = mybir.dt.float16  # "bf16" name kept; fp16 has 8x finer mantissa at same cost
    f32r_dt = mybir.dt.float32r
    AF = mybir.ActivationFunctionType
    OP = mybir.AluOpType
    odt = bf16 if bf16_out else f32

    nc = bacc.Bacc("TRN2", target_bir_lowering=False, debug=False)

    # Pin all activations to the single table set containing relu/ln/exp/
    # square so no act-table reloads are emitted mid-kernel.
    _PINNED_SET = "natural_log_exp_and_others"
    real_get_tables = bacc.get_activation_tables

    def pinned_get_tables(arch):
        tables = real_get_tables(arch)
        assert _PINNED_SET in tables
        return {
            name: (funcs if name == _PINNED_SET else set())
            for name, funcs in tables.items()
        }

    bacc.get_activation_tables = pinned_get_tables

    def register_const(val):
        th = nc.alloc_sbuf_tensor(f"const-float32-{val}", [128, 1], f32)
        nc.gpsimd.memset(th.ap(), val)
        nc.const_aps.aps[(f32, val)] = th.ap()

    register_const(MINIMUM_SCALE)
    register_const(1.0) if (f32, 1.0) not in nc.const_aps.aps else None
    nc.all_engine_barrier()

    if bench_internal:
        d_bench_in = nc.dram_tensor("bench_in", [1, 1], f32, kind="ExternalInput").ap()
        d_bench_out = nc.dram_tensor("bench_out", [1, 1], f32, kind="ExternalOutput").ap()
        d_data = nc.dram_tensor("data", [rows, t], f32).ap()
        d_mask = nc.dram_tensor("padding_mask", [rows, t], i32).ap()
        d_wts = nc.dram_tensor("weights", [rows, t], f32).ap()
        d_scaled = nc.dram_tensor("scaled", [rows, t], odt).ap()
        d_means = nc.dram_tensor("means", [rows, t], f32).ap()
        d_scale = nc.dram_tensor("scale", [rows, t], odt).ap()
    else:
        d_data = nc.dram_tensor("data", [rows, t], f32, kind="ExternalInput").ap()
        d_mask = nc.dram_tensor("padding_mask", [rows, t], i32, kind="ExternalInput").ap()
        d_wts = nc.dram_tensor("weights", [rows, t], f32, kind="ExternalInput").ap()
        d_scaled = nc.dram_tensor("scaled", [rows, t], odt, kind="ExternalOutput").ap()
        d_means = nc.dram_tensor("means", [rows, t], f32, kind="ExternalOutput").ap()
        d_scale = nc.dram_tensor("scale", [rows, t], odt, kind="ExternalOutput").ap()

    n_rt = rows // 128
    n_ch = t // w_chunk
    n_sub = w_chunk // sub
    nblk = sub // 128  # 128-blocks per subchunk

    with tile.TileContext(nc) as tc, ExitStack() as ctx:

        def pool(name, bufs, space="SBUF"):
            return ctx.enter_context(tc.tile_pool(name=name, bufs=bufs, space=space))

        consts = pool("consts", 1)
        pdata = pool("data", in_bufs + 2)   # d crosses phase1->phase2
        pmask = pool("mask", in_bufs)
        pwts = pool("wts", in_bufs)
        pw = pool("w", 3)
        pwT = pool("wT", eb)                # crosses phase1->phase2
        pwT_e = pool("wT_e", 2)             # f32 variant (exact chunks only)
        pwxT = pool("wxT", 3)
        pwxT_e = pool("wxT_e", 2)
        pdp = pool("dp", 2)
        pinv = pool("inv", eb)              # crosses phase1->phase2
        pmean = pool("mean", eb)            # crosses phase1->phase2
        pr = pool("r", eb)                  # crosses phase2->phase3
        pr2 = pool("r2", 3)
        psT = pool("sT", 3)
        pvar = pool("var", eb)              # crosses phase2->phase3
        plnv = pool("lnv", 2)
        pisc = pool("isc", 3)
        psc = pool("sc", 2)
        psd = pool("sd", 2)
        pcar = pool("car", 2 * n_rt + 2)
        ptr = pool("tr", tr_bufs, space="PSUM")      # transpose targets
        pmmA = pool("mmA", mm_bufs, space="PSUM")    # cw tiles
        pmmB = pool("mmB", mm_bufs + mmB_extra, space="PSUM")  # cwx tiles
        pmmC = pmmA if share_mmAC else pool("mmC", mm_bufs, space="PSUM")

        ident = consts.tile([128, 128], f32)
        make_identity(nc, ident[:])
        ident_h = consts.tile([128, 128], bf16, tag="ident_h")
        nc.scalar.copy(ident_h[:], ident[:])
        # master mask [128, 128*(2*nblk-1)]: [zeros*(nblk-1) | tri | ones*(nblk-1)]
        # block b of a subchunk uses window [:, (nblk-1-b)*128 : +sub]
        mw = 128 * (2 * nblk - 1)
        umask = consts.tile([128, mw], f32)
        z = 128 * (nblk - 1)
        if z:
            nc.gpsimd.memset(umask[:, 0:z], 0.0)
            nc.gpsimd.memset(umask[:, z + 128 : mw], 1.0)
        make_upper_triangular(nc, umask[:, z : z + 128], val=1.0, diag=True)
        umask_h = consts.tile([128, mw], bf16, tag="umask_h")
        nc.scalar.copy(umask_h[:], umask[:])

        if bench_internal:
            nc.sync.dma_start(d_bench_out[:, :], d_bench_in[:, :])
            zf = consts.tile([128, w_chunk], f32, tag="zf")
            nc.vector.memset(zf[:], 1.0)
            zi = consts.tile([128, w_chunk], i32, tag="zi")
            nc.vector.memset(zi[:], 1)
            for rt0 in range(n_rt):
                rsl0 = slice(rt0 * 128, (rt0 + 1) * 128)
                for ci0 in range(n_ch):
                    csl0 = bass.ts(ci0, w_chunk)
                    nc.sync.dma_start(d_data[rsl0, csl0], zf[:])
                    nc.sync.dma_start(d_wts[rsl0, csl0], zf[:])
                    nc.sync.dma_start(d_mask[rsl0, csl0], zi[:])

        carries = {}

        tiny = nc.gpsimd if pool_tiny else nc.vector
        state = {}

        # PE transposes into [128, 512] psum tiles
        def transp(src, dt_, idt):
            tiles = []
            for h in range(w_chunk // 512):
                ps = ptr.tile([128, 512], dt_, tag="tr")
                for b in range(4):
                    nc.tensor.transpose(
                        ps[:, b * 128 : (b + 1) * 128],
                        src[:, h * 512 + b * 128 : h * 512 + (b + 1) * 128],
                        idt[:],
                    )
                tiles.append(ps)
            return tiles

        # triangular matmuls; stationary = transposed data, moving = masks
        def tri_mm(mm, srcT, sc_i, mask):
            for b in range(nblk):
                lo = sc_i * sub + b * 128
                wnd = (nblk - 1 - b) * 128
                nc.tensor.matmul(
                    mm[:],
                    srcT[:, lo : lo + 128],
                    mask[:, wnd : wnd + sub],
                    start=(b == 0),
                    stop=(b == nblk - 1),
                )
            return mm

        loaded = {}

        def loads(rt, ci):
            if ci >= n_ch:
                return
            rsl = slice(rt * 128, (rt + 1) * 128)
            csl = bass.ts(ci, w_chunk)
            d = pdata.tile([128, w_chunk], f32)
            m = pmask.tile([128, w_chunk], i32)
            wt = pwts.tile([128, w_chunk], f32)
            nc.sync.dma_start(d[:], d_data[rsl, csl])
            nc.sync.dma_start(m[:], d_mask[rsl, csl])
            nc.sync.dma_start(wt[:], d_wts[rsl, csl])
            loaded[(rt, ci)] = (d, m, wt)

        wprep = {}

        def prep_w(rt, ci):
            """For non-exact chunks: w = wt*m (fp16) and its XBAR transpose,
            issued a chunk ahead so the DMA latency stays off the chain."""
            if ci >= n_ch or ci < f32_chunks or not dma_tr_w:
                return
            d, m, wt = loaded[(rt, ci)]
            w = pw.tile([128, w_chunk], bf16, tag="w0")
            eng_w = nc.gpsimd if w_on_pool else nc.vector
            eng_w.tensor_tensor(w[:], wt[:], m[:], OP.mult)
            wT = pwT.tile([128, w_chunk], bf16, tag="wT")
            nc.sync.dma_start_transpose(
                wT[:].rearrange("p (b c) -> p b c", c=128), w[:]
            )
            wprep[(rt, ci)] = wT

        def phase1(rt, ci):
            """w, transposes, wT, wxT, cw/cwx matmuls, inv, mean."""
            rsl = slice(rt * 128, (rt + 1) * 128)
            csl = bass.ts(ci, w_chunk)
            carm1, carB, carC = carries[rt]
            d, m, wt = loaded.pop((rt, ci))

            if dma_only:
                nc.sync.dma_start(d_scaled[rsl, csl], d[:].bitcast(odt) if bf16_out else d[:])
                nc.sync.dma_start(d_means[rsl, csl], d[:])
                nc.sync.dma_start(d_scale[rsl, csl], wt[:].bitcast(odt) if bf16_out else wt[:])
                return

            # Early chunks need exact (fp32) running sums for the mean path:
            # before ~512 accumulated terms there is no statistical rounding
            # cancellation and 16-bit noise in cw/cwx shows up as a mean
            # error amplified by 1/scale. Once the exact carry dominates,
            # fp16 matmuls are safe.
            exact = ci < f32_chunks
            sdt = f32 if exact else bf16
            smask = umask if exact else umask_h

            # w = weights * mask.  Non-exact chunks keep w in fp16 so the
            # transpose runs on the DMA XBAR (SBUF->SBUF, no PE / no copy).
            if (rt, ci) in wprep:
                wT = wprep.pop((rt, ci))
            else:
                w = pw.tile([128, w_chunk], f32, tag="w")
                eng_w = nc.gpsimd if w_on_pool else nc.vector
                eng_w.tensor_tensor(w[:], wt[:], m[:], OP.mult)
                wT = (pwT_e if exact else pwT).tile([128, w_chunk], sdt, tag="wT")
                wT_ps = transp(w, f32, ident)
                for h, ps in enumerate(wT_ps):
                    if copy_split and h % 2 == 1:
                        nc.vector.tensor_copy(wT[:, h * 512 : (h + 1) * 512], ps[:])
                    else:
                        nc.scalar.copy(wT[:, h * 512 : (h + 1) * 512], ps[:])
            # wxT = dT * wT computed directly in the transposed domain (DVE
            # reads the transposed-d psum tiles; no psum->sbuf copy needed)
            dT_ps = transp(d, f32, ident)
            wxT = (pwxT_e if exact else pwxT).tile([128, w_chunk], sdt, tag="wxT")
            for h, ps in enumerate(dT_ps):
                nc.vector.tensor_tensor(
                    wxT[:, h * 512 : (h + 1) * 512], ps[:],
                    wT[:, h * 512 : (h + 1) * 512], OP.mult,
                )

            dp = pdp.tile([128, w_chunk], f32)
            inv = pinv.tile([128, w_chunk], f32)
            mean = pmean.tile([128, w_chunk], f32)
            mmB_l = []  # (mmB, carry_prev) per subchunk
            for sc_i in range(n_sub):
                ssl = bass.ts(sc_i, sub)
                mmA = pmmA.tile([128, sub], f32, tag="mm")
                tri_mm(mmA, wT, sc_i, smask)
                mmB = pmmB.tile([128, sub], f32, tag="mm")
                tri_mm(mmB, wxT, sc_i, smask)
                mmB_l.append((mmB, carB))
                # inv = exp(-ln(relu(cw + (carry-1)) + 1)); with lninv,
                # non-exact chunks (cw >= 1 deterministically) skip the relu
                # clamp and take ln(cw) directly with the raw-cw carry bias.
                if lninv and not exact:
                    nc.scalar.activation(dp[:, ssl], mmA[:], AF.Ln, bias=carm1)
                elif dpcar and not exact:
                    # dp's previous last column IS (cw-1): use it as the relu
                    # bias directly (ACT-internal chain, no DVE tiny hop)
                    b_ap = carm1 if sc_i == 0 else dp[:, sc_i * sub - 1 : sc_i * sub]
                    nc.scalar.activation(dp[:, ssl], mmA[:], AF.Relu, bias=b_ap)
                else:
                    nc.scalar.activation(dp[:, ssl], mmA[:], AF.Relu, bias=carm1)
                if not (dpcar and not exact):
                    ncarm1 = pcar.tile([128, 1], f32, tag=f"A{rt}")
                    tiny.tensor_tensor(
                        ncarm1[:], mmA[:, sub - 1 : sub], carm1, OP.add
                    )
                    carm1 = ncarm1[:]
                # cwx carry update
                ncarB = pcar.tile([128, 1], f32, tag=f"B{rt}")
                if carbc_act:
                    nc.scalar.activation(
                        ncarB[:], mmB[:, sub - 1 : sub], AF.Identity,
                        bias=carB[:, 0:1],
                    )
                else:
                    tiny.tensor_tensor(
                        ncarB[:], mmB[:, sub - 1 : sub], carB[:, 0:1], OP.add
                    )
                carB = ncarB

            if dpcar and not exact:
                # one copy extracts the chunk-end carry (cw-1) from dp
                ncarm1 = pcar.tile([128, 1], f32, tag=f"A{rt}")
                tiny.tensor_copy(ncarm1[:], dp[:, w_chunk - 1 : w_chunk])
                carm1 = ncarm1[:]
            if not (lninv and not exact):
                nc.scalar.activation(dp[:], dp[:], AF.Ln, bias=1.0)
            nc.scalar.activation(inv[:], dp[:], AF.Exp, scale=-1.0)
            if lninv and exact and ci == f32_chunks - 1:
                # switch the A carry from (cum_w - 1) to raw cum_w
                carA_t = pcar.tile([128, 1], f32, tag=f"A{rt}")
                tiny.tensor_tensor(
                    carA_t[:], carm1, nc.const_aps.aps[(f32, 1.0)], OP.add
                )
                carm1 = carA_t[:]

            for sc_i in range(n_sub):
                ssl = bass.ts(sc_i, sub)
                mmB, cB = mmB_l[sc_i]
                nc.vector.scalar_tensor_tensor(
                    mean[:, ssl], mmB[:], cB[:, 0:1],
                    inv[:, ssl], OP.add, OP.mult,
                )
            if not no_stores:
                nc.sync.dma_start(d_means[rsl, csl], mean[:])
            carries[rt] = (carm1, carB, carC)
            state[rt] = (d, wT, inv, mean)

        def phase2(rt, ci):
            """r, r2, r2 transpose, sT, cs matmuls, var."""
            if dma_only:
                return
            carm1, carB, carC = carries[rt]
            d, wT, inv, mean = state[rt]
            rr = pr.tile([128, w_chunk], bf16)
            nc.vector.tensor_tensor(rr[:], d[:], mean[:], OP.subtract)
            r2 = pr2.tile([128, w_chunk], bf16)
            if r2_on_dve:
                nc.vector.tensor_tensor(r2[:], rr[:], rr[:], OP.mult)
            else:
                nc.scalar.activation(r2[:], rr[:], AF.Square)
            sT = psT.tile([128, w_chunk], bf16)
            if dma_tr_r2:
                r2T = pr2.tile([128, w_chunk], bf16, tag="r2T")
                nc.sync.dma_start_transpose(
                    r2T[:].rearrange("p (b c) -> p b c", c=128), r2[:]
                )
                for h in range(w_chunk // 512):
                    nc.vector.tensor_tensor(
                        sT[:, h * 512 : (h + 1) * 512],
                        r2T[:, h * 512 : (h + 1) * 512],
                        wT[:, h * 512 : (h + 1) * 512], OP.mult,
                    )
            else:
                r2T_ps = transp(r2, bf16, ident_h)
                for h, ps in enumerate(r2T_ps):
                    nc.vector.tensor_tensor(
                        sT[:, h * 512 : (h + 1) * 512], ps[:],
                        wT[:, h * 512 : (h + 1) * 512], OP.mult,
                    )

            var = pvar.tile([128, w_chunk], bf16)
            for sc_i in range(n_sub):
                ssl = bass.ts(sc_i, sub)
                mmC = pmmC.tile([128, sub], f32, tag="mm")
                tri_mm(mmC, sT, sc_i, umask_h)
                nc.vector.scalar_tensor_tensor(
                    var[:, ssl], mmC[:], carC[:, 0:1], inv[:, ssl],
                    OP.add, OP.mult,
                )
                ncarC = pcar.tile([128, 1], f32, tag=f"C{rt}")
                if carbc_act:
                    nc.scalar.activation(
                        ncarC[:], mmC[:, sub - 1 : sub], AF.Identity,
                        bias=carC[:, 0:1],
                    )
                else:
                    tiny.tensor_tensor(
                        ncarC[:], mmC[:, sub - 1 : sub], carC[:, 0:1], OP.add
                    )
                carC = ncarC
            carries[rt] = (carm1, carB, carC)
            state[rt] = state[rt] + (rr, var)

        def phase3(rt, ci):
            """lnv, isc, sc, sd, stores."""
            if dma_only:
                return
            rsl = slice(rt * 128, (rt + 1) * 128)
            csl = bass.ts(ci, w_chunk)
            d, wT, inv, mean, rr, var = state[rt]
            lnv = plnv.tile([128, w_chunk], bf16)
            nc.scalar.activation(lnv[:], var[:], AF.Ln, bias=MINIMUM_SCALE)
            isc = pisc.tile([128, w_chunk], bf16)
            nc.scalar.activation(isc[:], lnv[:], AF.Exp, scale=-0.5)
            sc = psc.tile([128, w_chunk], odt)
            if sc_on_dve:
                nc.vector.scalar_tensor_tensor(
                    sc[:], var[:], MINIMUM_SCALE, isc[:], OP.add, OP.mult
                )
            else:
                nc.scalar.activation(sc[:], lnv[:], AF.Exp, scale=0.5)
            sd = psd.tile([128, w_chunk], odt)
            (nc.gpsimd if sd_on_pool else nc.vector).tensor_tensor(sd[:], rr[:], isc[:], OP.mult)
            if not no_stores:
                nc.sync.dma_start(d_scale[rsl, csl], sc[:])
                nc.sync.dma_start(d_scaled[rsl, csl], sd[:])

        def emit_chunk(rt, ci):
            if (rt, ci) not in loaded:
                loads(rt, ci)
            phase1(rt, ci)
            phase2(rt, ci)
            phase3(rt, ci)

        def init_carries(rt):
            carm1 = pcar.tile([128, 1], f32, tag=f"A{rt}")
            nc.vector.memset(carm1[:], -1.0)
            carB = pcar.tile([128, 1], f32, tag=f"B{rt}")
            nc.vector.memset(carB[:], 0.0)
            carC = pcar.tile([128, 1], f32, tag=f"C{rt}")
            nc.vector.memset(carC[:], 0.0)
            carries[rt] = (carm1[:], carB, carC)

        def emit_all():
            carries.clear()
            state.clear()
            loaded.clear()
            if interleave:
                for rt in range(n_rt):
                    init_carries(rt)
                for rt in range(n_rt):
                    loads(rt, 0)
                for ci in range(n_ch):
                    for rt in range(n_rt):
                        loads(rt, ci + 1)
                    if ci < f32_chunks:
                        for rt in range(n_rt):
                            emit_chunk(rt, ci)
                        for rt in range(n_rt):
                            prep_w(rt, ci + 1)
                        continue
                    for rt in range(n_rt):
                        phase1(rt, ci)
                    for rt in range(n_rt):
                        prep_w(rt, ci + 1)
                    for rt in range(n_rt):
                        phase2(rt, ci)
                    for rt in range(n_rt):
                        phase3(rt, ci)
            else:
                for rt in range(n_rt):
                    init_carries(rt)
                    for ci in range(n_ch):
                        emit_chunk(rt, ci)

        if hw_reps:
            with tc.For_i(0, hw_reps):
                for rep in range(reps):
                    emit_all()
        else:
            for rep in range(reps):
                emit_all()

    try:
        nc.compile()
    finally:
        bacc.get_activation_tables = real_get_tables
    return nc


BEST_KW = {"sub": 512, "mmB_extra": 1, "tr_bufs": 1}


def _get_nc():
    if "nc" not in _CACHE:
        _CACHE["nc"] = _build_nc(ROWS, T, **BEST_KW)
    return _CACHE["nc"]


def _run(data, padding_mask, weights, trace=False):
    from concourse.bass_utils import run_bass_kernel_spmd

    nc = _get_nc()
    d = np.ascontiguousarray(np.asarray(data, dtype=np.float32).reshape(B * V, T))
    pm = np.ascontiguousarray(
        np.asarray(padding_mask, dtype=np.int32).reshape(B * V, T)
    )
    wt = np.ascontiguousarray(np.asarray(weights, dtype=np.float32).reshape(B * V, T))

    in_maps = [
        {
            "data": d[i * ROWS : (i + 1) * ROWS],
            "padding_mask": pm[i * ROWS : (i + 1) * ROWS],
            "weights": wt[i * ROWS : (i + 1) * ROWS],
        }
        for i in range(NCORES)
    ]
    res = run_bass_kernel_spmd(nc, in_maps, core_ids=list(range(NCORES)), trace=trace)

    def gather(name):
        return (
            np.concatenate(
                [np.asarray(res.results[i][name], dtype=np.float32) for i in range(NCORES)],
                axis=0,
            )
            .reshape(B, V, T)
        )

    return (gather("scaled"), gather("means"), gather("scale")), res


def kernel(data, padding_mask, weights):
    (scaled, means, scale), _ = _run(data, padding_mask, weights)
    return scaled, means, scale

